# revision 1
# baseline (speedup 1.0000x reference)
"""MoE kernel v5: 8-way F-split, all experts resident on every core.

Every core holds a distinct F/8 = 512-column slice of ALL 8 experts'
w1/w2 and processes ALL routed token columns (16384 = T*top_k) on that
slice; the 8 partial outputs are summed on host, then combined/scattered
with the router weights. Per-core PE work is exactly 16384 columns x 64
cycles regardless of the routing distribution - zero load imbalance.
(bf16 roofline: 16384 cols x 64 cyc / 2.4 GHz = 437 us; measured ~468.)

Token tiles are grouped per expert with balanced widths (count_e split
into near-equal tiles <= 512 wide, so every matmul free dim hides
LDWEIGHTS); the program's final tile is 128 wide with its own small
output tensor so the trailing DMA is tiny. x tiles are shared by all
cores; the expert id per tile is static in the program.

Schedule: mm1 runs one tile ahead of mm2 (software pipeline), so mm2
never waits on its own tile's gelu and the PE has work while w2[e0]
streams in. ~26 warm-up matmuls on a memset tile occupy the PE (and
warm the HAM clock gate) during the ~11 us the first x/w1 transfers
need to land. The 14 MB bulk weight stream rides the gpsimd SWDGE
queue, held back by WAR anchors until tile 1 is underway so it cannot
crowd the startup-critical transfers; x rides the sync HWDGE queue and
y the scalar HWDGE queue.

DRAM layouts per core (FL = F/8 = 512, FLO = FL/128 = 4):
  x   [n_tiles, 128, KO, CT]    bf16  x[t,p,ko,c] = xf[tok_c, ko*128+p]
  w1  [E, 128, FLO, KO, 128]    bf16  w1[e,p,fq,ko,c] =
                                        w1_e[ko*128+p, h*FL+fq*128+c]
  w2  [E, 128, FLO, D]          bf16  w2[e,p,fo,d] = w2_e[h*FL+fo*128+p, d]
  b1  [128, E*FLO]              f32   b1[p, e*FLO+fo] = b1_e[h*FL+fo*128+p]
  y   [n_tiles, 128, KO, CT]    bf16  partial (gelu(x@w1l+b1l) @ w2l)^T
  y2  [128, KO, TW_LAST]        bf16  same, for the final narrow tile
(h = the core's F-slice index, 0..7.)
"""

import numpy as np
import ml_dtypes

N_CORES = 8
D = 1024
F = 4096
E = 8
KO = D // 128
FL = F // N_CORES    # 512 local F columns per core
FLO = FL // 128      # 4 local f-chunks
CT = 512

BF16 = ml_dtypes.bfloat16

_NC_CACHE: dict[tuple, object] = {}
LAST_RESULTS = None


TW_LAST = 128   # width of the program's trailing narrow tiles
N_NARROW = 3    # how many trailing tiles are narrow


def _balanced_tiles(C, n_narrow=0):
    """Split C columns into near-equal tiles <= CT wide: [(off, w), ...].

    The final n_narrow tiles are exactly TW_LAST columns so the last
    full-width y evacuation has several tile-times of compute to hide
    under and the kernel's trailing DMAs are small.
    """
    if C <= 0:
        return []
    tail_n = n_narrow if C > n_narrow * TW_LAST + 256 else 0
    C2 = C - tail_n * TW_LAST
    tiles, off = [], 0
    if C2 > 0:
        n = (C2 + CT - 1) // CT
        base, rem = divmod(C2, n)
        widths = [base + 1] * rem + [base] * (n - rem)
        for w in widths:
            tiles.append((off, w))
            off += w
    for _ in range(tail_n):
        tiles.append((off, TW_LAST))
        off += TW_LAST
    return tiles


def _build(spec, b1_zero):
    import concourse.mybir as mybir
    from concourse import bacc
    from concourse.tile import TileContext

    fp32 = mybir.dt.float32
    bf16 = mybir.dt.bfloat16

    n_tiles = len(spec)
    e_first = spec[0][0]

    nc = bacc.Bacc(
        "TRN2", target_bir_lowering=False, debug=False, num_devices=N_CORES
    )
    x = nc.dram_tensor("x", [n_tiles, 128, KO, CT], bf16, kind="ExternalInput")
    w1 = nc.dram_tensor("w1", [E, 128, FLO, KO, 128], bf16, kind="ExternalInput")
    w2 = nc.dram_tensor("w2", [E, 128, FLO, D], bf16, kind="ExternalInput")
    b1 = nc.dram_tensor("b1", [128, E * FLO], fp32, kind="ExternalInput")
    n_nar = 0
    while n_nar < n_tiles and spec[n_tiles - 1 - n_nar][2] <= TW_LAST:
        n_nar += 1
    y = nc.dram_tensor("y", [n_tiles, 128, KO, CT], bf16, kind="ExternalOutput")
    y2 = nc.dram_tensor(
        "y2", [max(n_nar, 1), 128, KO, TW_LAST], bf16, kind="ExternalOutput"
    )

    with TileContext(nc) as tc:
        with (
            tc.tile_pool(name="wpool", bufs=1) as wpool,
            tc.tile_pool(name="xpool", bufs=2) as xpool,
            tc.tile_pool(name="hpool", bufs=3) as hpool,
            tc.tile_pool(name="ypool", bufs=4) as ypool,
            tc.tile_pool(name="ph", bufs=4, space="PSUM") as phpool,
            tc.tile_pool(name="py", bufs=4, space="PSUM") as pypool,
        ):
            w1_sb = wpool.tile([128, E, FLO, KO, 128], bf16)
            w2_sb = wpool.tile([128, E, FLO, D], bf16)
            b1_sb = wpool.tile([128, E * FLO], fp32)
            anchor = wpool.tile([128, 16], bf16)
            others = [e for e in range(E) if e != e_first]

            # Dummy activation up front so ACT_TABLE_LOAD (Gelu table,
            # ~1.3 us) runs at scalar t=0 instead of before tile0's
            # first real gelu.
            warm = wpool.tile([128, 1], fp32)
            nc.vector.memset(warm[:], 0.0)
            nc.scalar.activation(
                warm[:], warm[:], mybir.ActivationFunctionType.Gelu
            )

            # Startup-critical transfers. Any 128-row DMA has a ~8 us
            # per-queue latency floor (descriptor-rate-bound), so only
            # x[0] (sync) and w1[e0] (scalar) go first, in parallel;
            # w2[e0] follows on scalar (needed one tile-time later).
            # b1 is usually all-zeros here - memset it off the critical
            # path instead of paying a 128-descriptor DMA for 16 KB.
            # Startup: x[0] on the sync queue, w1[e0] then w2[e0] on
            # the scalar queue, in parallel; everything else is held
            # off the startup window.
            x_first = xpool.tile([128, KO, CT], bf16, tag="x_sb")
            nc.sync.dma_start(x_first[:, 0:4], x[0][:, 0:4])
            nc.sync.dma_start(x_first[:, 4:8], x[0][:, 4:8])
            for fq in range(FLO):
                nc.scalar.dma_start(w1_sb[:, e_first, fq], w1[e_first][:, fq])
            if b1_zero:
                nc.vector.memset(b1_sb[:], 0.0)
            else:
                nc.scalar.dma_start(b1_sb[:], b1[:])
            nc.scalar.dma_start(w2_sb[:, e_first], w2[e_first])

            # Warm-up matmuls on a memset tile: fill the time the first
            # x/w1 chunks need to land, so the PE is busy (and the HAM
            # clock warm) from ~7 us on. Sized to end as the DMAs land.
            wdummy = wpool.tile([128, CT], bf16)
            nc.vector.memset(wdummy[:], 0.0)
            for _ in range(26):
                ph = phpool.tile([128, CT], fp32, tag="ph")
                nc.tensor.matmul(
                    ph[:], lhsT=wdummy[:, 0:128], rhs=wdummy[:],
                    start=True, stop=True,
                )

            def mm1_tile(ti, e, tw, x_sb):
                h_sb = hpool.tile([128, FLO, CT], bf16)
                for fo in range(FLO):
                    ph = phpool.tile([128, CT], fp32, tag="ph")
                    for ko in range(KO):
                        nc.tensor.matmul(
                            ph[:, :tw],
                            lhsT=w1_sb[:, e, fo, ko, :],
                            rhs=x_sb[:, ko, :tw],
                            start=(ko == 0),
                            stop=(ko == KO - 1),
                        )
                    nc.scalar.activation(
                        h_sb[:, fo, :tw],
                        ph[:, :tw],
                        mybir.ActivationFunctionType.Gelu,
                        bias=b1_sb[:, e * FLO + fo : e * FLO + fo + 1],
                    )
                return h_sb

            def mm2_tile(ti, e, tw, h_sb):
                narrow = ti >= n_tiles - n_nar
                if narrow:
                    y_sb = ypool.tile([128, KO, TW_LAST], bf16, tag="y2_sb")
                else:
                    y_sb = ypool.tile([128, KO, CT], bf16, tag="y_sb")
                for do in range(KO):
                    py = pypool.tile([128, CT], fp32)
                    for fo in range(FLO):
                        nc.tensor.matmul(
                            py[:, :tw],
                            lhsT=w2_sb[:, e, fo, do * 128 : (do + 1) * 128],
                            rhs=h_sb[:, fo, :tw],
                            start=(fo == 0),
                            stop=(fo == FLO - 1),
                        )
                    nc.vector.tensor_copy(y_sb[:, do, :tw], py[:, :tw])
                # One DMA per tile (full width; pad columns carry
                # ignored stale data) on the scalar queue; sync carries
                # only the x stream. The trailing narrow tiles write
                # small dedicated outputs split across both queues, so
                # the last full-width y DMA has several tile-times of
                # compute to hide under and the final DMAs are tiny.
                if narrow:
                    idx = ti - (n_tiles - n_nar)
                    if ti == n_tiles - 1:
                        nc.scalar.dma_start(y2[idx][0:32], y_sb[0:32])
                        nc.sync.dma_start(y2[idx][32:64], y_sb[32:64])
                        nc.scalar.dma_start(y2[idx][64:96], y_sb[64:96])
                        nc.sync.dma_start(y2[idx][96:128], y_sb[96:128])
                    else:
                        nc.scalar.dma_start(y2[idx][0:64], y_sb[0:64])
                        nc.sync.dma_start(y2[idx][64:128], y_sb[64:128])
                else:
                    nc.scalar.dma_start(y[ti][:], y_sb[:])

            # Software pipeline: mm1 runs one tile ahead of mm2, so the
            # PE has mm1 work while w2[e0] is still streaming in, and
            # mm2 never waits on the gelu of its own tile.
            prev = None
            for ti, (e, off, tw) in enumerate(spec):
                if ti == 0:
                    x_sb = x_first
                else:
                    x_sb = xpool.tile([128, KO, CT], bf16, tag="x_sb")
                    nc.sync.dma_start(x_sb[:], x[ti])
                h_sb = mm1_tile(ti, e, tw, x_sb)
                anchor_ti = 1 if n_tiles > 1 else 0
                if ti == anchor_ti:
                    # WAR anchors: tiny reads of each pending weight
                    # region, chained after this tile's first h chunk,
                    # so the scheduler cannot hoist the 14 MB bulk
                    # weight stream (gpsimd queue) into the startup
                    # window. Expert e's weights are needed only once
                    # its tiles start.
                    nc.vector.tensor_copy(anchor[:, 0:1], h_sb[:, 0, 0:1])
                    for k, eo in enumerate(others):
                        nc.vector.tensor_add(
                            anchor[:, 1 + k : 2 + k],
                            w1_sb[:, eo, 0, 0, 0:1],
                            anchor[:, 0:1],
                        )
                        nc.vector.tensor_add(
                            anchor[:, 8 + k : 9 + k],
                            w2_sb[:, eo, 0, 0:1],
                            anchor[:, 0:1],
                        )
                    for eo in others:
                        nc.gpsimd.dma_start(w1_sb[:, eo], w1[eo])
                        nc.gpsimd.dma_start(w2_sb[:, eo], w2[eo])
                if prev is not None:
                    mm2_tile(*prev)
                prev = (ti, e, tw, h_sb)
            mm2_tile(*prev)

    nc.compile()
    return nc


def kernel(x, gate_w, w1, b1, w2, b2):
    from concourse.bass_utils import run_bass_kernel_spmd

    global LAST_RESULTS

    x = np.asarray(x, dtype=np.float32)
    gate_w = np.asarray(gate_w, dtype=np.float32)
    w1 = np.asarray(w1, dtype=np.float32)
    b1 = np.asarray(b1, dtype=np.float32)
    w2 = np.asarray(w2, dtype=np.float32)
    b2 = np.asarray(b2, dtype=np.float32)

    B, S, Din = x.shape
    assert Din == D and gate_w.shape == (D, E)
    T = B * S
    xf = x.reshape(T, D)

    # ---- Host router + dispatch ----
    logits = xf.astype(np.float64) @ gate_w.astype(np.float64)
    idx0 = np.argmax(logits, axis=1)
    rows = np.arange(T)
    v0 = logits[rows, idx0]
    l2 = logits.copy()
    l2[rows, idx0] = -np.inf
    idx1 = np.argmax(l2, axis=1)
    v1_ = l2[rows, idx1]
    e1 = np.exp(v1_ - v0)
    cw0 = 1.0 / (1.0 + e1)
    cw1 = e1 / (1.0 + e1)

    token_ids = []
    combine_w = []
    for e in range(E):
        sel0 = idx0 == e
        sel1 = idx1 == e
        ids = np.nonzero(sel0 | sel1)[0]
        w = np.where(sel0[ids], cw0[ids], cw1[ids])
        token_ids.append(ids)
        combine_w.append(w)

    spec = []
    for e in range(E):
        for off, tw in _balanced_tiles(
            len(token_ids[e]), n_narrow=(N_NARROW if e == E - 1 else 0)
        ):
            spec.append((e, off, tw))
    spec = tuple(spec)
    n_tiles = len(spec)

    b1_zero = bool(np.all(b1 == 0.0))
    key = (spec, b1_zero)
    if key not in _NC_CACHE:
        _NC_CACHE[key] = _build(spec, b1_zero)
    nc = _NC_CACHE[key]

    # ---- Shared x tiles; per-core weight slices ----
    xtiles = np.zeros((n_tiles, 128, KO, CT), dtype=BF16)
    for ti, (e, off, tw) in enumerate(spec):
        ids_seg = token_ids[e][off : off + tw]
        blk = xf[ids_seg].astype(BF16).reshape(tw, KO, 128).transpose(2, 1, 0)
        xtiles[ti, :, :, :tw] = blk
    xtiles = np.ascontiguousarray(xtiles)

    b1f = b1.astype(np.float32)
    in_maps = []
    for h in range(N_CORES):
        sl = slice(h * FL, (h + 1) * FL)
        w1c = np.stack(
            [
                w1[e][:, sl]
                .reshape(KO, 128, FLO, 128)
                .transpose(1, 2, 0, 3)
                for e in range(E)
            ]
        ).astype(BF16)  # [E, 128, FLO, KO, 128]
        w2c = np.stack(
            [w2[e][sl, :].reshape(FLO, 128, D).transpose(1, 0, 2) for e in range(E)]
        ).astype(BF16)  # [E, 128, FLO, D]
        b1c = np.stack(
            [b1f[e][sl].reshape(FLO, 128).T for e in range(E)], axis=1
        ).reshape(128, E * FLO)  # [128, E*FLO]
        in_maps.append(
            {
                "x": xtiles,
                "w1": np.ascontiguousarray(w1c),
                "w2": np.ascontiguousarray(w2c),
                "b1": np.ascontiguousarray(b1c),
            }
        )

    res = run_bass_kernel_spmd(nc, in_maps, core_ids=list(range(N_CORES)))
    LAST_RESULTS = res

    # ---- Host: sum the 8 F-slice partials, combine, scatter ----
    ysum = res.results[0]["y"].astype(np.float32)
    y2sum = res.results[0]["y2"].astype(np.float32)
    for h in range(1, N_CORES):
        ysum += res.results[h]["y"].astype(np.float32)
        y2sum += res.results[h]["y2"].astype(np.float32)

    n_nar = 0
    while n_nar < n_tiles and spec[n_tiles - 1 - n_nar][2] <= TW_LAST:
        n_nar += 1

    out = np.zeros((T, D), dtype=np.float32)
    for ti, (e, off, tw) in enumerate(spec):
        ids_seg = token_ids[e][off : off + tw]
        cw_seg = combine_w[e][off : off + tw].astype(np.float32)
        if ti >= n_tiles - n_nar:
            idx = ti - (n_tiles - n_nar)
            yt = y2sum[idx, :, :, :tw].transpose(2, 1, 0).reshape(tw, D)
        else:
            yt = ysum[ti, :, :, :tw].transpose(2, 1, 0).reshape(tw, D)
        out[ids_seg] += cw_seg[:, None] * (yt + b2[e])

    return out.reshape(B, S, D)

